# revision 1
# baseline (speedup 1.0000x reference)
"""Trainium2 8-core tensor-parallel transformer layer — v8.

On top of the v2 pipelined design (chunked AllGathers in separate DRAM
bounce tensors, batched LN row math, LN stats folded into producer loops):
- Warmup dummy AllGather absorbs the first-collective init cost.
- bf16 LN1 stats matmuls; bf16 residual stream (ln_in/attn_sb/mlp_sb).
- Dense projection chunks interleaved into the attention loop (batch b's
  dense runs during batch b+1's attention, sharing the ctx PSUM tags).
- Attention software-pipelined by one kt iteration: all four heads'
  scores+exp are emitted a full iteration ahead of the ctx matmuls.
- All three stat AllReduces split into token-halves; AR3 for the first
  half fires mid-attention (its dense chunks finished early), so the
  LN3 -> AR2 -> LN2 -> x2-AllGather chain for half 0 starts the moment
  attention ends, and LN4 of half 0 overlaps 4hh of half 1.
"""

import os
import sys

sys.path.insert(0, "/opt/trn_rl_repo")
os.environ.setdefault("MYCRO_LOCAL_CACHE", "1")
os.environ.setdefault("JAX_PLATFORMS", "cpu,axon")

import numpy as np
import ml_dtypes

import concourse.bass as bass
import concourse.mybir as mybir
import concourse.tile as tile
from concourse import bacc
from concourse.bass_utils import run_bass_kernel_spmd

F32 = mybir.dt.float32
BF16 = mybir.dt.bfloat16
AF = mybir.ActivationFunctionType
ALU = mybir.AluOpType

P = 128
B, S, H, NH = 4, 1024, 2048, 32
HD = H // NH
T = B * S
NC = 8
HPC = NH // NC                 # 4 heads/core
DC = H // NC                   # 256
FC = 4 * H // NC               # 1024
F4 = 4 * H                     # 8192
TC = 512
NTC = T // TC                  # 8
NFC = H // P                   # 16
EPS = 1e-5
RG = [list(range(NC))]

bf16 = ml_dtypes.bfloat16


def _causal_block_status(mask2d):
    mt = mask2d.T
    status = {}
    for kt in range(S // P):
        for qc in range(S // TC):
            blk = mt[kt * P:(kt + 1) * P, qc * TC:(qc + 1) * TC]
            if np.all(blk == 0):
                status[(kt, qc)] = "skip"
            elif np.all(blk == 1):
                status[(kt, qc)] = "full"
            else:
                status[(kt, qc)] = "masked"
    return status


def _evict(nc, dst, ps, bias_ap, zero_bias):
    if zero_bias:
        nc.scalar.activation(dst, ps, AF.Copy)
    else:
        nc.scalar.activation(dst, ps, AF.Identity, bias=bias_ap)


def build_program(block_status, zero_bv=True, zero_bias=True):
    nc = bacc.Bacc("TRN2", target_bir_lowering=False, debug=False,
                   num_devices=NC)

    def register_const_ap(dtype, value):
        t = nc.alloc_sbuf_tensor(f"const-{dtype.name}-{value}", [128, 1], dtype)
        nc.gpsimd.memset(t.ap(), value)
        nc.const_aps.aps[(dtype, value)] = t.ap()

    register_const_ap(F32, EPS)
    register_const_ap(F32, float(1.0 / np.sqrt(HD)))
    nc.all_engine_barrier()

    # ---------------- DRAM I/O (same contract as v1) ----------------
    h_ln1 = nc.dram_tensor("h_ln1", [H, TC], F32, kind="ExternalInput")
    h_res = nc.dram_tensor("h_res", [DC, T], F32, kind="ExternalInput")
    ln1_w = nc.dram_tensor("ln1_w", [H, 1], F32, kind="ExternalInput")
    ln1_b = nc.dram_tensor("ln1_b", [H, 1], F32, kind="ExternalInput")
    ln2_w = nc.dram_tensor("ln2_w", [DC, 1], F32, kind="ExternalInput")
    ln2_b = nc.dram_tensor("ln2_b", [DC, 1], F32, kind="ExternalInput")
    ln3_w = nc.dram_tensor("ln3_w", [DC, 1], F32, kind="ExternalInput")
    ln3_b = nc.dram_tensor("ln3_b", [DC, 1], F32, kind="ExternalInput")
    ln4_w = nc.dram_tensor("ln4_w", [DC, 1], F32, kind="ExternalInput")
    ln4_b = nc.dram_tensor("ln4_b", [DC, 1], F32, kind="ExternalInput")
    w_qkv = nc.dram_tensor("w_qkv", [H, 3 * DC], BF16, kind="ExternalInput")
    b_qk = nc.dram_tensor("b_qk", [2 * DC, 1], F32, kind="ExternalInput")
    b_v = nc.dram_tensor("b_v", [1, DC], F32, kind="ExternalInput")
    w_dense = nc.dram_tensor("w_dense", [H, DC], BF16, kind="ExternalInput")
    b_dense = nc.dram_tensor("b_dense", [DC, 1], F32, kind="ExternalInput")
    w_h4h = nc.dram_tensor("w_h4h", [H, FC], BF16, kind="ExternalInput")
    b_h4h = nc.dram_tensor("b_h4h", [FC, 1], F32, kind="ExternalInput")
    w_4hh = nc.dram_tensor("w_4hh", [F4, DC], BF16, kind="ExternalInput")
    b_4hh = nc.dram_tensor("b_4hh", [DC, 1], F32, kind="ExternalInput")
    maskT = nc.dram_tensor("maskT", [S, S], BF16, kind="ExternalInput")
    out_ext = nc.dram_tensor("out", [DC, T], F32, kind="ExternalOutput")

    masked_blocks = sorted(k for k, v in block_status.items() if v == "masked")
    mask_slot = {blk: i for i, blk in enumerate(masked_blocks)}

    with tile.TileContext(nc) as tc:
        with tc.tile_pool(name="const", bufs=1) as const, \
             tc.tile_pool(name="resid", bufs=1) as resid, \
             tc.tile_pool(name="dram", bufs=1, space="DRAM") as dram:

            # ---------- constants ----------
            ones_f = const.tile([P, 1], F32)
            nc.vector.memset(ones_f[:, :], 1.0)
            ones_bf = const.tile([P, 1], BF16)
            nc.vector.memset(ones_bf[:, :], 1.0)
            ones_rows_bf = const.tile([P, P], BF16)
            nc.vector.memset(ones_rows_bf[:, :], 1.0)

            ln1w_sb = const.tile([P, NFC], F32)
            ln1b_sb = const.tile([P, NFC], F32)
            for fc in range(NFC):
                nc.sync.dma_start(out=ln1w_sb[:, fc:fc + 1],
                                  in_=ln1_w[fc * P:(fc + 1) * P, 0:1])
                nc.sync.dma_start(out=ln1b_sb[:, fc:fc + 1],
                                  in_=ln1_b[fc * P:(fc + 1) * P, 0:1])

            cpack = const.tile([P, 28], F32)
            _cofs = [0]

            def load_cols(t, ncols=2):
                base = _cofs[0]
                _cofs[0] += ncols
                for m in range(ncols):
                    nc.sync.dma_start(out=cpack[:, base + m:base + m + 1],
                                      in_=t[m * P:(m + 1) * P, 0:1])
                return cpack[:, base:base + ncols]

            ln2w_sb = load_cols(ln2_w)
            ln2b_sb = load_cols(ln2_b)
            ln3w_sb = load_cols(ln3_w)
            ln3b_sb = load_cols(ln3_b)
            ln4w_sb = load_cols(ln4_w)
            ln4b_sb = load_cols(ln4_b)
            bdense_sb = load_cols(b_dense)
            b4hh_sb = load_cols(b_4hh)
            bqk_sb = load_cols(b_qk, 4)
            bh4h_sb = load_cols(b_h4h, 8)

            if not zero_bv:
                bv_row = const.tile([1, DC], F32)
                nc.sync.dma_start(out=bv_row[:, :], in_=b_v[0:1, :])
                bv_b = const.tile([P, DC], F32)
                nc.gpsimd.partition_broadcast(bv_b[:, :], bv_row[:, :])

            if masked_blocks:
                mask_sb = const.tile([P, len(masked_blocks) * TC], BF16)
                for (kt, qc), i in mask_slot.items():
                    nc.sync.dma_start(
                        out=mask_sb[:, i * TC:(i + 1) * TC],
                        in_=maskT[kt * P:(kt + 1) * P, qc * TC:(qc + 1) * TC])

            # ---------- residents ----------
            ln_in = [resid.tile([P, T], BF16, name=f"ln_in{m}")
                     for m in range(2)]
            attn_sb = [resid.tile([P, T], BF16, tag="colsAM", bufs=2,
                                  name=f"attn_sb{m}") for m in range(2)]

            # ---------- DRAM bounces (chunked) ----------
            # x1: 2 feature-halves [128, 8*TC] each
            ag_x1_in = [dram.tile([P, 8 * TC], BF16, name=f"agx1i{h}")
                        for h in range(2)]
            ag_x1_out = [dram.tile([NC * P, 8 * TC], BF16,
                                   addr_space="Shared", name=f"agx1o{h}")
                         for h in range(2)]
            # ctx: per-batch chunks, free = qc*2*TC... chunk layout
            # [128, 2048]: (qc, pair) -> qc*1024 + pair*512
            ag_ctx_in = [dram.tile([P, 2 * S], BF16, name=f"agcxi{b}")
                         for b in range(B)]
            ag_ctx_out = [dram.tile([NC * P, 2 * S], BF16,
                                    addr_space="Shared", name=f"agcxo{b}")
                          for b in range(B)]
            # x2: 2 token-halves, free = t8r*1024 + m*512 (t8r in 0..3)
            ag_x2_in = [dram.tile([P, 2 * 1024], BF16, name=f"agx2i{h}")
                        for h in range(4)]
            ag_x2_out = [dram.tile([NC * P, 2 * 1024], BF16,
                                   addr_space="Shared", name=f"agx2o{h}")
                         for h in range(4)]
            # inter: per-t8 chunks, free = m*512 (m in 0..7)
            ag_int_in = [dram.tile([P, 8 * TC], BF16, name=f"agini{k}")
                         for k in range(NTC)]
            ag_int_out = [dram.tile([NC * P, 8 * TC], BF16,
                                    addr_space="Shared", name=f"agino{k}")
                          for k in range(NTC)]
            ar3h_in = [dram.tile([2, T // 2], F32, name=f"ar3i{i}")
                       for i in range(2)]
            ar3h_out = [dram.tile([2, T // 2], F32, addr_space="Shared",
                                  name=f"ar3o{i}") for i in range(2)]
            ar2h_in = [dram.tile([2, T // 2], F32, name=f"ar2i{i}")
                       for i in range(2)]
            ar2h_out = [dram.tile([2, T // 2], F32, addr_space="Shared",
                                  name=f"ar2o{i}") for i in range(2)]
            ar4h_in = [dram.tile([2, T // 2], F32, name=f"ar4i{i}")
                       for i in range(2)]
            ar4h_out = [dram.tile([2, T // 2], F32, addr_space="Shared",
                                  name=f"ar4o{i}") for i in range(2)]

            warm_in = dram.tile([1, 64], BF16, name="warm_in")
            warm_out = dram.tile([NC, 64], BF16, addr_space="Shared",
                                 name="warm_out")
            warm_sb = const.tile([1, 64], BF16)
            nc.vector.memset(warm_sb[:, :], 0.0)
            nc.sync.dma_start(out=warm_in[:, :], in_=warm_sb[:, :])
            nc.gpsimd.collective_compute(
                "AllGather", ALU.bypass, replica_groups=RG,
                ins=[warm_in[:, :].opt()], outs=[warm_out[:, :].opt()])

            # =========================================================
            # Phase A: LN1 -> x1 (bf16) -> 2 half AllGathers
            # =========================================================
            with tc.tile_pool(name="ph_a", bufs=1) as pha, \
                 tc.tile_pool(name="ph_a_ps", bufs=2, space="PSUM") as phaps:
                h1 = [pha.tile([P, TC], F32, name=f"h1_{fc}")
                      for fc in range(NFC)]
                for fc in range(NFC):
                    nc.sync.dma_start(out=h1[fc][:, :],
                                      in_=h_ln1[fc * P:(fc + 1) * P, :])
                ps_s = phaps.tile([1, TC], F32, name="ps_s")
                ps_q = phaps.tile([1, TC], F32, name="ps_q")
                for fc in range(NFC):
                    h1b = pha.tile([P, TC], BF16, tag="h1b", bufs=3,
                                   name="h1b")
                    nc.vector.tensor_copy(h1b[:, :], h1[fc][:, :])
                    nc.tensor.matmul(ps_s[:, :], ones_bf[:, 0:1], h1b[:, :],
                                     start=(fc == 0), stop=(fc == NFC - 1))
                    sq = pha.tile([P, TC], BF16, tag="sq", bufs=3, name="sq")
                    nc.vector.tensor_mul(sq[:, :], h1b[:, :], h1b[:, :])
                    nc.tensor.matmul(ps_q[:, :], ones_bf[:, 0:1], sq[:, :],
                                     start=(fc == 0), stop=(fc == NFC - 1))
                mu = pha.tile([1, TC], F32)
                m2 = pha.tile([1, TC], F32)
                var = pha.tile([1, TC], F32)
                sd = pha.tile([1, TC], F32)
                a_row = pha.tile([1, TC], F32)
                b2_row = pha.tile([1, TC], F32)
                nc.vector.tensor_scalar_mul(mu[:, :], ps_s[:, :], 1.0 / H)
                nc.vector.tensor_scalar_mul(m2[:, :], ps_q[:, :], 1.0 / H)
                nc.vector.tensor_mul(var[:, :], mu[:, :], mu[:, :])
                nc.vector.tensor_sub(var[:, :], m2[:, :], var[:, :])
                nc.scalar.activation(sd[:, :], var[:, :], AF.Sqrt, bias=EPS)
                nc.vector.reciprocal(a_row[:, :], sd[:, :])
                nc.vector.tensor_mul(b2_row[:, :], mu[:, :], a_row[:, :])
                nc.vector.tensor_scalar_mul(b2_row[:, :], b2_row[:, :], -1.0)
                a_b = pha.tile([P, TC], F32)
                b2_b = pha.tile([P, TC], F32)
                nc.gpsimd.partition_broadcast(a_b[:, :], a_row[:, :])
                nc.gpsimd.partition_broadcast(b2_b[:, :], b2_row[:, :])
                x1h = [pha.tile([P, 8 * TC], BF16, name=f"x1h{h}")
                       for h in range(2)]
                for fc in range(NFC):
                    t1 = pha.tile([P, TC], F32, tag="t1", bufs=3, name="t1")
                    nc.vector.tensor_mul(t1[:, :], h1[fc][:, :], a_b[:, :])
                    nc.vector.tensor_add(t1[:, :], t1[:, :], b2_b[:, :])
                    hh, fr = fc // 8, fc % 8
                    nc.vector.tensor_scalar(
                        x1h[hh][:, fr * TC:(fr + 1) * TC], t1[:, :],
                        ln1w_sb[:, fc:fc + 1], ln1b_sb[:, fc:fc + 1],
                        ALU.mult, ALU.add)
                    if fr == 7:
                        nc.sync.dma_start(out=ag_x1_in[hh][:, :],
                                          in_=x1h[hh][:, :])

            for hh in range(2):
                nc.gpsimd.collective_compute(
                    "AllGather", ALU.bypass, replica_groups=RG,
                    ins=[ag_x1_in[hh][:, :].opt()],
                    outs=[ag_x1_out[hh][:, :].opt()])

            # =========================================================
            # Phase B: QKV (consumes x1 halves as they arrive)
            # =========================================================
            phd_w_cm = tc.tile_pool(name="ph_d_w", bufs=1)
            phdw = phd_w_cm.__enter__()
            phd_cm = tc.tile_pool(name="ph_d", bufs=1)
            phd = phd_cm.__enter__()
            attn_res_cm = tc.tile_pool(name="attn_res", bufs=1)
            attn_res = attn_res_cm.__enter__()
            qT2 = attn_res.tile([P, 2 * T], BF16)
            kT2 = attn_res.tile([P, 2 * T], BF16)
            v_sb = attn_res.tile([P, (T // P) * DC], BF16)
            with tc.tile_pool(name="ph_b_w", bufs=1) as phbw, \
                 tc.tile_pool(name="ph_b", bufs=2) as phb, \
                 tc.tile_pool(name="ph_b_ps", bufs=3, space="PSUM") as phbps:
                wq_all = phbw.tile([P, NFC * 3 * DC], BF16, name="wq_all")
                for fc in range(NFC):
                    nc.sync.dma_start(
                        out=wq_all[:, fc * 3 * DC:(fc + 1) * 3 * DC],
                        in_=w_qkv[fc * P:(fc + 1) * P, :])
                for t8 in range(NTC):
                    x1c = [phb.tile([P, 8 * TC], BF16, tag=f"x1c{h}",
                                    name=f"x1c{h}") for h in range(2)]
                    for hh in range(2):
                        nc.sync.dma_start(
                            out=x1c[hh][:, :],
                            in_=ag_x1_out[hh][t8 * P:(t8 + 1) * P, :])

                    def xs(fc, lo, sz):
                        hh, fr = fc // 8, fc % 8
                        return x1c[hh][:, fr * TC + lo: fr * TC + lo + sz]

                    for m in range(4):
                        ps = phbps.tile([P, TC], F32, tag="qk", name="ps_qk")
                        for fc in range(NFC):
                            nc.tensor.matmul(
                                ps[:, :],
                                wq_all[:, fc * 3 * DC + m * P:
                                       fc * 3 * DC + (m + 1) * P],
                                xs(fc, 0, TC),
                                start=(fc == 0), stop=(fc == NFC - 1))
                        dst = qT2 if m < 2 else kT2
                        pair = m % 2
                        off = pair * T + t8 * TC
                        _evict(nc, dst[:, off:off + TC], ps[:, :],
                               bqk_sb[:, m:m + 1], zero_bias)
                    for tt in range(TC // P):
                        psv = phbps.tile([P, DC], F32, tag="v", name="ps_v")
                        for fc in range(NFC):
                            nc.tensor.matmul(
                                psv[:, :], xs(fc, tt * P, P),
                                wq_all[:, fc * 3 * DC + 2 * DC:
                                       fc * 3 * DC + 3 * DC],
                                start=(fc == 0), stop=(fc == NFC - 1))
                        ttg = t8 * (TC // P) + tt
                        voff = ttg * DC
                        if zero_bv:
                            nc.scalar.activation(v_sb[:, voff:voff + DC],
                                                 psv[:, :], AF.Copy)
                        else:
                            nc.vector.tensor_add(v_sb[:, voff:voff + DC],
                                                 psv[:, :], bv_b[:, :])

            # =========================================================
            # Phase C: attention; ctxF free = (qc*1024 + pair*512) per b
            # =========================================================
            ctx_cm = tc.tile_pool(name="ctx_pool", bufs=1)
            ctx_pool = ctx_cm.__enter__()
            ctxF = [ctx_pool.tile([P, 2 * S], BF16, name=f"ctxF{b}")
                    for b in range(B)]
            wd_all = phdw.tile([P, NFC * DC], BF16, name="wd_all")
            for fc in range(NFC):
                nc.sync.dma_start(out=wd_all[:, fc * DC:(fc + 1) * DC],
                                  in_=w_dense[fc * P:(fc + 1) * P, :])
            with tc.tile_pool(name="ph_c", bufs=1) as phc, \
                 tc.tile_pool(name="ph_c_ps", bufs=1, space="PSUM") as phcps:

                def dense_chunk(t8):
                    b_, qc_ = t8 // 2, t8 % 2
                    cx_t = [phd.tile([P, TC], BF16, tag="cx", bufs=18,
                                     name=f"cx{q}") for q in range(NFC)]
                    for c8 in range(NC):
                        for p2 in range(2):
                            fc = c8 * 2 + p2
                            nc.sync.dma_start(
                                out=cx_t[fc][:, :],
                                in_=ag_ctx_out[b_][
                                    c8 * P:(c8 + 1) * P,
                                    qc_ * 2 * TC + p2 * TC:
                                    qc_ * 2 * TC + (p2 + 1) * TC])
                    dps = [phcps.tile([P, TC], F32, tag=f"ctx{m}", bufs=1,
                                      name=f"ps_d{m}") for m in range(2)]
                    for fc in range(NFC):
                        for m in range(2):
                            nc.tensor.matmul(
                                dps[m][:, :],
                                wd_all[:, fc * DC + m * P:
                                       fc * DC + (m + 1) * P],
                                cx_t[fc][:, :],
                                start=(fc == 0), stop=(fc == NFC - 1))
                    for m in range(2):
                        _evict(nc, attn_sb[m][:, t8 * TC:(t8 + 1) * TC],
                               dps[m][:, :], bdense_sb[:, m:m + 1],
                               zero_bias)
                    _stats_t8(nc, phd, phcps, attn_sb, t8,
                              ar3h_in[t8 // 4], ones_bf, stag="s", qtag="s",
                              sbufs=4, slot=t8 % 4)

                for b in range(B):
                    for qc in range(S // TC):
                        ctx_ps = [phcps.tile([P, TC], F32, tag=f"ctx{p}",
                                             bufs=1, name=f"ctx_ps{p}")
                                  for p in range(2)]
                        den_ps = phcps.tile([P, TC], F32, tag="den",
                                            bufs=1, name="den_ps")
                        kts = [kt for kt in range(S // P)
                               if block_status[(kt, qc)] != "skip"]
                        nkt = len(kts)

                        def emit_scores(ki):
                            kt = kts[ki]
                            st = block_status[(kt, qc)]
                            es = []
                            for h in range(HPC):
                                pair, rho = h // 2, h % 2
                                ps_s = phcps.tile([P, TC], F32, tag="s",
                                                  bufs=4, name="ps_s")
                                qoff = pair * T + b * S + qc * TC
                                koff = pair * T + b * S + kts[ki] * P
                                nc.tensor.matmul(
                                    ps_s[:, :],
                                    kT2[rho * HD:(rho + 1) * HD,
                                        koff:koff + P],
                                    qT2[rho * HD:(rho + 1) * HD,
                                        qoff:qoff + TC],
                                    start=True, stop=True)
                                e = phc.tile([P, TC], BF16, tag="e", bufs=10,
                                             name="e")
                                nc.scalar.activation(e[:, :], ps_s[:, :],
                                                     AF.Exp,
                                                     scale=1.0 / np.sqrt(HD))
                                if st == "masked":
                                    i = mask_slot[(kt, qc)]
                                    nc.vector.tensor_mul(
                                        e[:, :], e[:, :],
                                        mask_sb[:, i * TC:(i + 1) * TC])
                                es.append(e)
                            return es

                        def emit_ctx(ki, es):
                            kt = kts[ki]
                            ttg = b * (S // P) + kt
                            for h in range(HPC):
                                pair, rho = h // 2, h % 2
                                nc.tensor.matmul(
                                    ctx_ps[pair][rho * HD:(rho + 1) * HD, :],
                                    v_sb[:, ttg * DC + h * HD:
                                         ttg * DC + (h + 1) * HD],
                                    es[h][:, :],
                                    start=(ki == 0), stop=(ki == nkt - 1))
                                nc.tensor.matmul(
                                    den_ps[32 * h:32 * h + 1, :],
                                    ones_bf[:, 0:1], es[h][:, :],
                                    start=(ki == 0), stop=(ki == nkt - 1),
                                    tile_position=(0, 32 * h))

                        prev = emit_scores(0)
                        for ki in range(1, nkt):
                            cur = emit_scores(ki)
                            emit_ctx(ki - 1, prev)
                            prev = cur
                        emit_ctx(nkt - 1, prev)
                        # batched reciprocal of all 4 head denominators
                        rd = phc.tile([P, TC], F32, tag="rd", bufs=2,
                                      name="rd")
                        rd_bf = phc.tile([P, TC], BF16, tag="rd_bf", bufs=2,
                                         name="rd_bf")
                        nc.vector.reciprocal(rd[:, :], den_ps[:, :])
                        nc.vector.tensor_copy(rd_bf[:, :], rd[:, :])
                        for h in range(HPC):
                            pair, rho = h // 2, h % 2
                            r32 = slice(32 * h, 32 * h + 1)
                            ps_b = phcps.tile([P, TC], F32, tag="s", bufs=4,
                                              name="ps_b")
                            nc.tensor.matmul(ps_b[:, :],
                                             ones_rows_bf[r32, :],
                                             rd_bf[r32, :], start=True,
                                             stop=True,
                                             tile_position=(32 * h, 0))
                            rd_b = phc.tile([P, TC], F32, tag="rd_b", bufs=2,
                                            name="rd_b")
                            nc.vector.tensor_copy(rd_b[:, :], ps_b[:, :])
                            off = qc * 2 * TC + pair * TC
                            hs = slice(rho * HD, (rho + 1) * HD)
                            nc.vector.tensor_mul(ctxF[b][hs, off:off + TC],
                                                 ctx_ps[pair][hs, :],
                                                 rd_b[hs, :])
                        if b > 0:
                            dense_chunk(2 * (b - 1) + qc)
                        if b == 2 and qc == 1:
                            nc.gpsimd.collective_compute(
                                "AllReduce", ALU.add, replica_groups=RG,
                                ins=[ar3h_in[0][:, :].opt()],
                                outs=[ar3h_out[0][:, :].opt()])
                    nc.sync.dma_start(out=ag_ctx_in[b][:, :],
                                      in_=ctxF[b][:, :])
                    nc.gpsimd.collective_compute(
                        "AllGather", ALU.bypass, replica_groups=RG,
                        ins=[ag_ctx_in[b][:, :].opt()],
                        outs=[ag_ctx_out[b][:, :].opt()])
                for qc in range(2):
                    dense_chunk(6 + qc)
                nc.gpsimd.collective_compute(
                    "AllReduce", ALU.add, replica_groups=RG,
                    ins=[ar3h_in[1][:, :].opt()],
                    outs=[ar3h_out[1][:, :].opt()])
            ctx_cm.__exit__(None, None, None)
            attn_res_cm.__exit__(None, None, None)
            phd_ps_cm = tc.tile_pool(name="ph_d_ps", bufs=1, space="PSUM")
            phdps = phd_ps_cm.__enter__()

            if True:
                with tc.tile_pool(name="ph_d2", bufs=2) as phd2:
                    x2 = [phd2.tile([P, 2 * 1024], BF16, tag=f"x2_{h}",
                                    bufs=1, name=f"x2_{h}") for h in range(4)]
                    for half in range(2):
                        ab3 = _ln_rows_batch(nc, phd2, ar3h_out[half],
                                             f"ln3h{half}", nrows=4)
                        for t8 in range(4 * half, 4 * half + 4):
                            a_b, b2_b = _ln_bcast(nc, phd2, ab3, t8 % 4)
                            for m in range(2):
                                hres = phd2.tile([P, TC], F32, tag="hres",
                                                 bufs=8, name="hres")
                                nc.sync.dma_start(
                                    out=hres[:, :],
                                    in_=h_res[m * P:(m + 1) * P,
                                              t8 * TC:(t8 + 1) * TC])
                                sl = slice(t8 * TC, (t8 + 1) * TC)
                                t1 = phd2.tile([P, TC], F32, tag="t1",
                                               name="t1")
                                t2 = phd2.tile([P, TC], F32, tag="t2",
                                               name="t2")
                                nc.vector.tensor_mul(t1[:, :],
                                                     attn_sb[m][:, sl],
                                                     a_b[:, :])
                                nc.vector.tensor_add(t1[:, :], t1[:, :],
                                                     b2_b[:, :])
                                nc.vector.tensor_scalar(t2[:, :], t1[:, :],
                                                        ln3w_sb[:, m:m + 1],
                                                        ln3b_sb[:, m:m + 1],
                                                        ALU.mult, ALU.add)
                                nc.vector.tensor_add(ln_in[m][:, sl],
                                                     t2[:, :], hres[:, :])
                            _stats_t8(nc, phd2, phdps, ln_in, t8,
                                      ar2h_in[half], ones_bf, slot=t8 % 4)
                        nc.gpsimd.collective_compute(
                            "AllReduce", ALU.add, replica_groups=RG,
                            ins=[ar2h_in[half][:, :].opt()],
                            outs=[ar2h_out[half][:, :].opt()])
                        ab2 = _ln_rows_batch(nc, phd2, ar2h_out[half],
                                             f"ln2h{half}", nrows=4)
                        for t8 in range(4 * half, 4 * half + 4):
                            a_b, b2_b = _ln_bcast(nc, phd2, ab2, t8 % 4)
                            t8r = t8 % 4
                            for m in range(2):
                                sl = slice(t8 * TC, (t8 + 1) * TC)
                                t1 = phd2.tile([P, TC], F32, tag="t1",
                                               name="t1")
                                nc.vector.tensor_mul(t1[:, :],
                                                     ln_in[m][:, sl],
                                                     a_b[:, :])
                                nc.vector.tensor_add(t1[:, :], t1[:, :],
                                                     b2_b[:, :])
                                qq, tq = t8 // 2, t8 % 2
                                nc.vector.tensor_scalar(
                                    x2[qq][:, tq * 1024 + m * TC:
                                           tq * 1024 + (m + 1) * TC],
                                    t1[:, :], ln2w_sb[:, m:m + 1],
                                    ln2b_sb[:, m:m + 1], ALU.mult, ALU.add)
                            if t8 % 2 == 1:
                                qq = t8 // 2
                                nc.sync.dma_start(out=ag_x2_in[qq][:, :],
                                                  in_=x2[qq][:, :])
                                nc.gpsimd.collective_compute(
                                    "AllGather", ALU.bypass,
                                    replica_groups=RG,
                                    ins=[ag_x2_in[qq][:, :].opt()],
                                    outs=[ag_x2_out[qq][:, :].opt()])

            phd_cm.__exit__(None, None, None)
            phd_w_cm.__exit__(None, None, None)

            # =========================================================
            # Phase E+F: h4h+gelu -> per-t8 inter AGs -> 4hh (+LN4 stats)
            # one shared PSUM pool: h4h groups (4) + 4hh (2) + stats (2)
            # =========================================================
            with tc.tile_pool(name="ph_e_w", bufs=1) as phew, \
                 tc.tile_pool(name="ph_ef", bufs=1) as phef, \
                 tc.tile_pool(name="ph_ef_ps", bufs=1, space="PSUM") as pheps:
                whp_cm = tc.tile_pool(name="ph_wh", bufs=1)
                whp = whp_cm.__enter__()
                wh_all = whp.tile([P, NFC * FC], BF16, name="wh_all")
                for fc in range(NFC):
                    nc.sync.dma_start(out=wh_all[:, fc * FC:(fc + 1) * FC],
                                      in_=w_h4h[fc * P:(fc + 1) * P, :])
                w4_all = phew.tile([P, (F4 // P) * DC], BF16,
                                   name="w4_all")
                for j in range(F4 // P):
                    nc.sync.dma_start(out=w4_all[:, j * DC:(j + 1) * DC],
                                      in_=w_4hh[j * P:(j + 1) * P, :])
                mlp_sb = [resid.tile([P, T], BF16, tag="colsAM", bufs=2,
                                     name=f"mlp_sb{m}") for m in range(2)]
                # h4h producer per t8 chunk
                for t8 in range(NTC):
                    hh, t8r = t8 // 2, t8 % 2
                    x2c_all = phef.tile([P, NFC * TC], BF16, tag="x2c",
                                        bufs=2, name="x2c_all")
                    for c8 in range(NC):
                        for m2 in range(2):
                            fc = c8 * 2 + m2
                            nc.sync.dma_start(
                                out=x2c_all[:, fc * TC:(fc + 1) * TC],
                                in_=ag_x2_out[hh][c8 * P:(c8 + 1) * P,
                                                  t8r * 1024 + m2 * TC:
                                                  t8r * 1024 + (m2 + 1) * TC])
                    for g in range(4):
                        it = phef.tile([P, 2 * TC], BF16, tag="it", bufs=2,
                                       name="it")
                        ps = [pheps.tile([P, TC], F32, tag=f"h{mi}", bufs=1,
                                         name=f"ps_h{mi}") for mi in range(2)]
                        for fc in range(NFC):
                            for mi in range(2):
                                m = g * 2 + mi
                                nc.tensor.matmul(
                                    ps[mi][:, :],
                                    wh_all[:, fc * FC + m * P:
                                           fc * FC + (m + 1) * P],
                                    x2c_all[:, fc * TC:(fc + 1) * TC],
                                    start=(fc == 0), stop=(fc == NFC - 1))
                        for mi in range(2):
                            nc.scalar.activation(
                                it[:, mi * TC:(mi + 1) * TC], ps[mi][:, :],
                                AF.Gelu_apprx_tanh,
                                bias=bh4h_sb[:, g * 2 + mi:g * 2 + mi + 1])
                        nc.sync.dma_start(
                            out=ag_int_in[t8][:, g * 2 * TC:(g + 1) * 2 * TC],
                            in_=it[:, :])
                    nc.gpsimd.collective_compute(
                        "AllGather", ALU.bypass, replica_groups=RG,
                        ins=[ag_int_in[t8][:, :].opt()],
                        outs=[ag_int_out[t8][:, :].opt()])
                whp_cm.__exit__(None, None, None)
                # 4hh consumer per t8 chunk
                for t8 in range(NTC):
                    ps = [pheps.tile([P, TC], F32, tag=f"f{m}", bufs=1,
                                     name=f"ps_f{m}") for m in range(2)]
                    for j in range(F4 // P):
                        c8, m8 = j // 8, j % 8
                        i4 = phef.tile([P, TC], BF16, tag="i4", bufs=12,
                                       name="i4")
                        nc.sync.dma_start(
                            out=i4[:, :],
                            in_=ag_int_out[t8][c8 * P:(c8 + 1) * P,
                                               m8 * TC:(m8 + 1) * TC])
                        for m in range(2):
                            nc.tensor.matmul(
                                ps[m][:, :],
                                w4_all[:, j * DC + m * P:
                                       j * DC + (m + 1) * P],
                                i4[:, :],
                                start=(j == 0), stop=(j == F4 // P - 1))
                    for m in range(2):
                        _evict(nc, mlp_sb[m][:, t8 * TC:(t8 + 1) * TC],
                               ps[m][:, :], b4hh_sb[:, m:m + 1], zero_bias)
                    _stats_t8(nc, phef, pheps, mlp_sb, t8,
                              ar4h_in[t8 // 4], ones_bf, slot=t8 % 4)
                    if t8 == 3 or t8 == 7:
                        ih = t8 // 4
                        nc.gpsimd.collective_compute(
                            "AllReduce", ALU.add, replica_groups=RG,
                            ins=[ar4h_in[ih][:, :].opt()],
                            outs=[ar4h_out[ih][:, :].opt()])

                # LN4 normalize + final residual -> out
                with tc.tile_pool(name="ph_f2", bufs=1) as phf2:
                    ab4h = [None, None]
                    for t8 in range(NTC):
                        if t8 % 4 == 0:
                            ab4h[t8 // 4] = _ln_rows_batch(
                                nc, phf2, ar4h_out[t8 // 4], f"ln4h{t8 // 4}",
                                nrows=4)
                        a_b, b2_b = _ln_bcast(nc, phf2, ab4h[t8 // 4],
                                              t8 % 4)
                        for m in range(2):
                            sl = slice(t8 * TC, (t8 + 1) * TC)
                            t1 = phf2.tile([P, TC], F32, tag="t1", name="t1")
                            t2 = phf2.tile([P, TC], F32, tag="t2", name="t2")
                            nc.vector.tensor_mul(t1[:, :], mlp_sb[m][:, sl],
                                                 a_b[:, :])
                            nc.vector.tensor_add(t1[:, :], t1[:, :],
                                                 b2_b[:, :])
                            nc.vector.tensor_scalar(t2[:, :], t1[:, :],
                                                    ln4w_sb[:, m:m + 1],
                                                    ln4b_sb[:, m:m + 1],
                                                    ALU.mult, ALU.add)
                            ot = phf2.tile([P, TC], F32, tag="ot", name="ot")
                            nc.vector.tensor_add(ot[:, :], t2[:, :],
                                                 ln_in[m][:, sl])
                            nc.sync.dma_start(
                                out=out_ext[m * P:(m + 1) * P,
                                            t8 * TC:(t8 + 1) * TC],
                                in_=ot[:, :])
            phd_ps_cm.__exit__(None, None, None)

    nc.compile()
    return nc


def _stats_t8(nc, pool, pspool, rows, t8, ar_in, ones_bf,
              stag="st_s", qtag="st_q", sbufs=1, slot=None):
    """Sum & sumsq over the 256 local features of token-chunk t8 (bf16)."""
    if slot is None:
        slot = t8
    ps_s = pspool.tile([1, TC], F32, tag=stag, bufs=sbufs, name="ps_s")
    ps_q = pspool.tile([1, TC], F32, tag=qtag, bufs=sbufs, name="ps_q")
    sl = slice(t8 * TC, (t8 + 1) * TC)
    osl = slice(slot * TC, (slot + 1) * TC)
    for m in range(2):
        nc.tensor.matmul(ps_s[:, :], ones_bf[:, 0:1], rows[m][:, sl],
                         start=(m == 0), stop=(m == 1))
    for m in range(2):
        sq = pool.tile([P, TC], BF16, tag="sq", bufs=2, name="sq")
        nc.vector.tensor_mul(sq[:, :], rows[m][:, sl], rows[m][:, sl])
        nc.tensor.matmul(ps_q[:, :], ones_bf[:, 0:1], sq[:, :],
                         start=(m == 0), stop=(m == 1))
    tmp_s = pool.tile([1, TC], F32, tag="tmp_s", bufs=1, name="tmp_s")
    tmp_q = pool.tile([1, TC], F32, tag="tmp_q", bufs=1, name="tmp_q")
    nc.vector.tensor_copy(tmp_s[:, :], ps_s[:, :])
    nc.vector.tensor_copy(tmp_q[:, :], ps_q[:, :])
    nc.sync.dma_start(out=ar_in[0:1, osl], in_=tmp_s[:, :])
    nc.sync.dma_start(out=ar_in[1:2, osl], in_=tmp_q[:, :])


def _ln_rows_batch(nc, pool, ar_out, name, nrows=8):
    """Batched LN row math on [nrows,TC] tiles, one reciprocal total."""
    s8 = pool.tile([nrows, TC], F32, tag="lnrb_s8", bufs=1, name=f"{name}_s8")
    q8 = pool.tile([nrows, TC], F32, tag="lnrb_q8", bufs=1, name=f"{name}_q8")
    nc.sync.dma_start(out=s8[:, :], in_=ar_out[0:1, :])
    nc.sync.dma_start(out=q8[:, :], in_=ar_out[1:2, :])
    mu = pool.tile([nrows, TC], F32, tag="lnrb_mu", bufs=1, name=f"{name}_mu")
    m2 = pool.tile([nrows, TC], F32, tag="lnrb_m2", bufs=1, name=f"{name}_m2")
    var = pool.tile([nrows, TC], F32, tag="lnrb_var", bufs=1,
                    name=f"{name}_var")
    sd = pool.tile([nrows, TC], F32, tag="lnrb_sd", bufs=1, name=f"{name}_sd")
    a8 = pool.tile([nrows, TC], F32, tag="lnrb_a8", bufs=1, name=f"{name}_a8")
    b28 = pool.tile([nrows, TC], F32, tag="lnrb_b28", bufs=1,
                    name=f"{name}_b28")
    nc.vector.tensor_scalar_mul(mu[:, :], s8[:, :], 1.0 / H)
    nc.vector.tensor_scalar_mul(m2[:, :], q8[:, :], 1.0 / H)
    nc.vector.tensor_mul(var[:, :], mu[:, :], mu[:, :])
    nc.vector.tensor_sub(var[:, :], m2[:, :], var[:, :])
    nc.scalar.activation(sd[:, :], var[:, :], AF.Sqrt, bias=EPS)
    nc.vector.reciprocal(a8[:, :], sd[:, :])
    nc.vector.tensor_mul(b28[:, :], mu[:, :], a8[:, :])
    nc.vector.tensor_scalar_mul(b28[:, :], b28[:, :], -1.0)
    return a8, b28


def _ln_bcast(nc, pool, ab, t8):
    """Extract row t8 from the batched (a8,b28) and broadcast to [P,TC]."""
    a8, b28 = ab
    a_row = pool.tile([1, TC], F32, tag="a_row", name="a_row")
    b2_row = pool.tile([1, TC], F32, tag="b2_row", name="b2_row")
    nc.sync.dma_start(out=a_row[:, :], in_=a8[t8:t8 + 1, :])
    nc.sync.dma_start(out=b2_row[:, :], in_=b28[t8:t8 + 1, :])
    a_b = pool.tile([P, TC], F32, tag="a_b", name="a_b")
    b2_b = pool.tile([P, TC], F32, tag="b2_b", name="b2_b")
    nc.gpsimd.partition_broadcast(a_b[:, :], a_row[:, :])
    nc.gpsimd.partition_broadcast(b2_b[:, :], b2_row[:, :])
    return a_b, b2_b


# ----------------------------------------------------------------------
_cache = {}


def _get_program(mask_np, zero_bv, zero_bias):
    key = (mask_np.tobytes(), zero_bv, zero_bias)
    kh = hash(key)
    if kh not in _cache:
        _cache[kh] = build_program(_causal_block_status(mask_np), zero_bv,
                                   zero_bias)
    return _cache[kh]


def kernel(hidden_states, mask, ln1_w, ln1_b, w_qkv, b_qkv, w_dense, b_dense,
           ln3_w, ln3_b, ln2_w, ln2_b, w_h4h, b_h4h, w_4hh, b_4hh,
           ln4_w, ln4_b):
    hidden_states = np.asarray(hidden_states, np.float32)
    mask2d = np.asarray(mask, np.float32).reshape(S, S)
    w_qkv = np.asarray(w_qkv, np.float32)
    b_qkv = np.asarray(b_qkv, np.float32)
    w_dense = np.asarray(w_dense, np.float32)
    w_h4h = np.asarray(w_h4h, np.float32)
    w_4hh = np.asarray(w_4hh, np.float32)

    zero_bv = bool(np.all(b_qkv[2 * H:] == 0.0))
    zero_bias = bool(np.all(b_qkv[:2 * H] == 0.0)
                     and np.all(np.asarray(b_dense) == 0.0)
                     and np.all(np.asarray(b_4hh) == 0.0))
    prog = _get_program(mask2d, zero_bv, zero_bias)

    hT = np.ascontiguousarray(hidden_states.reshape(T, H).T)
    maskT_bf = np.ascontiguousarray(mask2d.T).astype(bf16)

    in_maps = []
    for c in range(NC):
        qs = slice(c * DC, (c + 1) * DC)
        wq_c = np.concatenate([w_qkv[:, c * DC:(c + 1) * DC],
                               w_qkv[:, H + c * DC:H + (c + 1) * DC],
                               w_qkv[:, 2 * H + c * DC:2 * H + (c + 1) * DC]],
                              axis=1)
        b_qk_c = np.concatenate([b_qkv[c * DC:(c + 1) * DC],
                                 b_qkv[H + c * DC:H + (c + 1) * DC]])
        b_v_c = b_qkv[2 * H + c * DC:2 * H + (c + 1) * DC]
        im = {
            "h_ln1": np.ascontiguousarray(hT[:, c * TC:(c + 1) * TC]),
            "h_res": np.ascontiguousarray(hT[qs, :]),
            "ln1_w": np.asarray(ln1_w, np.float32).reshape(H, 1),
            "ln1_b": np.asarray(ln1_b, np.float32).reshape(H, 1),
            "ln2_w": np.asarray(ln2_w, np.float32)[qs].reshape(DC, 1),
            "ln2_b": np.asarray(ln2_b, np.float32)[qs].reshape(DC, 1),
            "ln3_w": np.asarray(ln3_w, np.float32)[qs].reshape(DC, 1),
            "ln3_b": np.asarray(ln3_b, np.float32)[qs].reshape(DC, 1),
            "ln4_w": np.asarray(ln4_w, np.float32)[qs].reshape(DC, 1),
            "ln4_b": np.asarray(ln4_b, np.float32)[qs].reshape(DC, 1),
            "w_qkv": np.ascontiguousarray(wq_c).astype(bf16),
            "b_qk": np.ascontiguousarray(b_qk_c).reshape(2 * DC, 1),
            "b_v": np.ascontiguousarray(b_v_c).reshape(1, DC),
            "w_dense": np.ascontiguousarray(w_dense[:, qs]).astype(bf16),
            "b_dense": np.asarray(b_dense, np.float32)[qs].reshape(DC, 1),
            "w_h4h": np.ascontiguousarray(
                w_h4h[:, c * FC:(c + 1) * FC]).astype(bf16),
            "b_h4h": np.asarray(b_h4h, np.float32)[
                c * FC:(c + 1) * FC].reshape(FC, 1),
            "w_4hh": np.ascontiguousarray(w_4hh[:, qs]).astype(bf16),
            "b_4hh": np.asarray(b_4hh, np.float32)[qs].reshape(DC, 1),
            "maskT": maskT_bf,
        }
        in_maps.append(im)

    res = run_bass_kernel_spmd(prog, in_maps, core_ids=list(range(NC)))
    outT = np.concatenate([res.results[c]["out"] for c in range(NC)], axis=0)
    return np.ascontiguousarray(outT.T).reshape(B, S, H).astype(np.float32)



# revision 9
# speedup vs baseline: 1.2432x; 1.2432x over previous
"""Trainium2 8-core transformer layer — v9.

Hybrid tensor-parallel/data-parallel layout:
- LN1 is data-parallel over token chunks (each core owns 512 tokens);
  x1 is AllGathered (2 feature-half chunks, bf16).
- QKV is column-parallel (4 heads/core), attention runs on local heads
  over all tokens.  The softmax denominator is folded into the ctx
  matmul by appending a ones-column to V (M=65 output rows).
- One small AllToAll (2 MB) re-shards the attention context from
  head-sharded to token-sharded.  Everything after — dense projection,
  LN3+residual, LN2, h4h+gelu, 4hh, LN4+residual — is computed fully
  locally per core for its own 512 tokens with full weights streamed
  from HBM (w_dense/w_h4h/w_4hh replicated, chunk-streamed), so there
  are no further collectives and no stats AllReduces.
"""

import os
import sys

sys.path.insert(0, "/opt/trn_rl_repo")
os.environ.setdefault("MYCRO_LOCAL_CACHE", "1")
os.environ.setdefault("JAX_PLATFORMS", "cpu,axon")

import numpy as np
import ml_dtypes

import concourse.bass as bass
import concourse.mybir as mybir
import concourse.tile as tile
from concourse import bacc
from concourse.bass_utils import run_bass_kernel_spmd

F32 = mybir.dt.float32
BF16 = mybir.dt.bfloat16
AF = mybir.ActivationFunctionType
ALU = mybir.AluOpType

P = 128
B, S, H, NH = 4, 1024, 2048, 32
HD = H // NH
T = B * S
NC = 8
HPC = NH // NC                 # 4 heads/core
DC = H // NC                   # 256 qkv features/core
F4 = 4 * H                     # 8192
TC = 512                       # tokens per core / token chunk
NTC = T // TC                  # 8
KB = H // P                    # 16 feature blocks of H
KB4 = F4 // P                  # 64 feature blocks of 4H
EPS = 1e-5
VW = HD + 1                    # v slot width per head (v | ones)
RG = [list(range(NC))]

bf16 = ml_dtypes.bfloat16


def _causal_block_status(mask2d):
    mt = mask2d.T
    status = {}
    for kt in range(S // P):
        for qc in range(S // TC):
            blk = mt[kt * P:(kt + 1) * P, qc * TC:(qc + 1) * TC]
            if np.all(blk == 0):
                status[(kt, qc)] = "skip"
            elif np.all(blk == 1):
                status[(kt, qc)] = "full"
            else:
                status[(kt, qc)] = "masked"
    return status


def build_program(block_status, zero_bv=True, ln_triv=True):
    nc = bacc.Bacc("TRN2", target_bir_lowering=False, debug=False,
                   num_devices=NC)

    def register_const_ap(dtype, value):
        t = nc.alloc_sbuf_tensor(f"const-{dtype.name}-{value}", [128, 1], dtype)
        nc.gpsimd.memset(t.ap(), value)
        nc.const_aps.aps[(dtype, value)] = t.ap()

    register_const_ap(F32, EPS)
    register_const_ap(F32, float(1.0 / np.sqrt(HD)))
    nc.all_engine_barrier()

    # ---------------- DRAM I/O ----------------
    h_ln1 = nc.dram_tensor("h_ln1", [H, TC], F32, kind="ExternalInput")
    ln1_w = nc.dram_tensor("ln1_w", [H, 1], F32, kind="ExternalInput")
    ln1_b = nc.dram_tensor("ln1_b", [H, 1], F32, kind="ExternalInput")
    ln2_w = nc.dram_tensor("ln2_w", [H, 1], F32, kind="ExternalInput")
    ln2_b = nc.dram_tensor("ln2_b", [H, 1], F32, kind="ExternalInput")
    ln3_w = nc.dram_tensor("ln3_w", [H, 1], F32, kind="ExternalInput")
    ln3_b = nc.dram_tensor("ln3_b", [H, 1], F32, kind="ExternalInput")
    ln4_w = nc.dram_tensor("ln4_w", [H, 1], F32, kind="ExternalInput")
    ln4_b = nc.dram_tensor("ln4_b", [H, 1], F32, kind="ExternalInput")
    w_qkv = nc.dram_tensor("w_qkv", [H, 3 * DC], BF16, kind="ExternalInput")
    b_qk = nc.dram_tensor("b_qk", [2 * DC, 1], F32, kind="ExternalInput")
    b_v = nc.dram_tensor("b_v", [1, DC], F32, kind="ExternalInput")
    w_dense = nc.dram_tensor("w_dense", [H, H], BF16, kind="ExternalInput")
    b_dense = nc.dram_tensor("b_dense", [H, 1], F32, kind="ExternalInput")
    w_h4h = nc.dram_tensor("w_h4h", [H, F4], BF16, kind="ExternalInput")
    b_h4h = nc.dram_tensor("b_h4h", [F4, 1], F32, kind="ExternalInput")
    w_4hh = nc.dram_tensor("w_4hh", [F4, H], BF16, kind="ExternalInput")
    b_4hh = nc.dram_tensor("b_4hh", [H, 1], F32, kind="ExternalInput")
    maskT = nc.dram_tensor("maskT", [S, S], BF16, kind="ExternalInput")
    out_ext = nc.dram_tensor("out", [H, TC], F32, kind="ExternalOutput")

    masked_blocks = sorted(k for k, v in block_status.items() if v == "masked")
    mask_slot = {blk: i for i, blk in enumerate(masked_blocks)}

    with tile.TileContext(nc) as tc:
        with tc.tile_pool(name="const", bufs=1) as const, \
             tc.tile_pool(name="resid", bufs=1) as resid, \
             tc.tile_pool(name="dram", bufs=1, space="DRAM") as dram:

            # ---------- constants ----------
            ones_bf = const.tile([P, 1], BF16)
            nc.vector.memset(ones_bf[:, :], 1.0)

            ln1w_sb = const.tile([P, KB], F32)
            ln1b_sb = const.tile([P, KB], F32)
            if not ln_triv:
                for fc in range(KB):
                    nc.sync.dma_start(out=ln1w_sb[:, fc:fc + 1],
                                      in_=ln1_w[fc * P:(fc + 1) * P, 0:1])
                    nc.sync.dma_start(out=ln1b_sb[:, fc:fc + 1],
                                      in_=ln1_b[fc * P:(fc + 1) * P, 0:1])

            cpack = const.tile([P, 8 * KB + 4 + KB4], F32)
            _cofs = [0]

            def load_cols(t, ncols):
                base = _cofs[0]
                _cofs[0] += ncols
                for m in range(ncols):
                    nc.sync.dma_start(out=cpack[:, base + m:base + m + 1],
                                      in_=t[m * P:(m + 1) * P, 0:1])
                return cpack[:, base:base + ncols]

            bdense_sb = load_cols(b_dense, KB)
            b4hh_sb = load_cols(b_4hh, KB)
            bqk_sb = load_cols(b_qk, 4)
            bh4h_sb = load_cols(b_h4h, KB4)
            if not ln_triv:
                ln2w_sb = load_cols(ln2_w, KB)
                ln2b_sb = load_cols(ln2_b, KB)
                ln3w_sb = load_cols(ln3_w, KB)
                ln3b_sb = load_cols(ln3_b, KB)
                ln4w_sb = load_cols(ln4_w, KB)
                ln4b_sb = load_cols(ln4_b, KB)

            if not zero_bv:
                bv_row = const.tile([1, DC], F32)
                nc.sync.dma_start(out=bv_row[:, :], in_=b_v[0:1, :])
                bv_b = const.tile([P, DC], F32)
                nc.gpsimd.partition_broadcast(bv_b[:, :], bv_row[:, :])

            if masked_blocks:
                mask_sb = const.tile([P, len(masked_blocks) * TC], BF16)
                for (kt, qc), i in mask_slot.items():
                    nc.sync.dma_start(
                        out=mask_sb[:, i * TC:(i + 1) * TC],
                        in_=maskT[kt * P:(kt + 1) * P, qc * TC:(qc + 1) * TC])

            # ---------- residents ----------
            h_bf = resid.tile([P, KB * TC], BF16, name="h_bf")
            ln_in = resid.tile([P, KB * TC], BF16, name="ln_in")
            x2 = resid.tile([P, KB * TC], BF16, name="x2")

            # ---------- DRAM bounces ----------
            ag_x1_in = [dram.tile([P, 8 * TC], BF16, name=f"agx1i{h}")
                        for h in range(2)]
            ag_x1_out = [dram.tile([NC * P, 8 * TC], BF16,
                                   addr_space="Shared", name=f"agx1o{h}")
                         for h in range(2)]
            a2a_in = dram.tile([H, TC], BF16, name="a2ai")
            a2a_out = dram.tile([H, TC], BF16, name="a2ao")

            warm_in = dram.tile([1, 64], BF16, name="warm_in")
            warm_out = dram.tile([NC, 64], BF16, addr_space="Shared",
                                 name="warm_out")
            warm2_in = dram.tile([NC, 64], BF16, name="warm2_in")
            warm2_out = dram.tile([NC, 64], BF16, name="warm2_out")
            warm_sb = const.tile([1, 64], BF16)
            nc.vector.memset(warm_sb[:, :], 0.0)
            nc.sync.dma_start(out=warm_in[:, :], in_=warm_sb[:, :])
            for c in range(NC):
                nc.sync.dma_start(out=warm2_in[c:c + 1, :], in_=warm_sb[:, :])
            nc.gpsimd.collective_compute(
                "AllGather", ALU.bypass, replica_groups=RG,
                ins=[warm_in[:, :].opt()], outs=[warm_out[:, :].opt()])
            nc.gpsimd.collective_compute(
                "AllToAll", ALU.bypass, replica_groups=RG,
                ins=[warm2_in[:, :].opt()], outs=[warm2_out[:, :].opt()])

            # =========================================================
            # Phase A: LN1 (local tokens) -> x1 bf16 -> 2 half AGs
            # =========================================================
            with tc.tile_pool(name="ph_a", bufs=1) as pha, \
                 tc.tile_pool(name="ph_a_ps", bufs=2, space="PSUM") as phaps:
                ps_s = phaps.tile([1, TC], F32, name="ps_s")
                ps_q = phaps.tile([1, TC], F32, name="ps_q")
                for fc in range(KB):
                    hsl = slice(fc * TC, (fc + 1) * TC)
                    h1 = pha.tile([P, TC], F32, tag="h1", bufs=3, name="h1")
                    nc.sync.dma_start(out=h1[:, :],
                                      in_=h_ln1[fc * P:(fc + 1) * P, :])
                    nc.vector.tensor_copy(h_bf[:, hsl], h1[:, :])
                    nc.tensor.matmul(ps_s[:, :], ones_bf[:, 0:1],
                                     h_bf[:, hsl],
                                     start=(fc == 0), stop=(fc == KB - 1))
                    sq = pha.tile([P, TC], BF16, tag="sq", bufs=3, name="sq")
                    nc.vector.tensor_mul(sq[:, :], h_bf[:, hsl], h_bf[:, hsl])
                    nc.tensor.matmul(ps_q[:, :], ones_bf[:, 0:1], sq[:, :],
                                     start=(fc == 0), stop=(fc == KB - 1))
                mu = pha.tile([1, TC], F32)
                m2 = pha.tile([1, TC], F32)
                var = pha.tile([1, TC], F32)
                sd = pha.tile([1, TC], F32)
                a_row = pha.tile([1, TC], F32)
                b2_row = pha.tile([1, TC], F32)
                nc.vector.tensor_scalar_mul(mu[:, :], ps_s[:, :], 1.0 / H)
                nc.vector.tensor_scalar_mul(m2[:, :], ps_q[:, :], 1.0 / H)
                nc.vector.tensor_mul(var[:, :], mu[:, :], mu[:, :])
                nc.vector.tensor_sub(var[:, :], m2[:, :], var[:, :])
                nc.scalar.activation(sd[:, :], var[:, :], AF.Sqrt, bias=EPS)
                nc.vector.reciprocal(a_row[:, :], sd[:, :])
                nc.vector.tensor_mul(b2_row[:, :], mu[:, :], a_row[:, :])
                nc.vector.tensor_scalar_mul(b2_row[:, :], b2_row[:, :], -1.0)
                a_b = pha.tile([P, TC], F32)
                b2_b = pha.tile([P, TC], F32)
                nc.gpsimd.partition_broadcast(a_b[:, :], a_row[:, :])
                nc.gpsimd.partition_broadcast(b2_b[:, :], b2_row[:, :])
                for fc in range(KB):
                    hsl = slice(fc * TC, (fc + 1) * TC)
                    t1 = pha.tile([P, TC], F32, tag="t1", bufs=3, name="t1")
                    nc.vector.tensor_mul(t1[:, :], h_bf[:, hsl], a_b[:, :])
                    x1t = pha.tile([P, TC], BF16, tag="x1t", bufs=3,
                                   name="x1t")
                    if ln_triv:
                        nc.vector.tensor_add(x1t[:, :], t1[:, :], b2_b[:, :])
                    else:
                        nc.vector.tensor_add(t1[:, :], t1[:, :], b2_b[:, :])
                        nc.vector.tensor_scalar(
                            x1t[:, :], t1[:, :],
                            ln1w_sb[:, fc:fc + 1], ln1b_sb[:, fc:fc + 1],
                            ALU.mult, ALU.add)
                    hh, fr = fc // 8, fc % 8
                    nc.sync.dma_start(
                        out=ag_x1_in[hh][:, fr * TC:(fr + 1) * TC],
                        in_=x1t[:, :])

            for hh in range(2):
                nc.gpsimd.collective_compute(
                    "AllGather", ALU.bypass, replica_groups=RG,
                    ins=[ag_x1_in[hh][:, :].opt()],
                    outs=[ag_x1_out[hh][:, :].opt()])

            # =========================================================
            # Phase B: QKV (column-parallel, consumes x1 halves)
            # =========================================================
            attn_res_cm = tc.tile_pool(name="attn_res", bufs=1)
            attn_res = attn_res_cm.__enter__()
            qT2 = attn_res.tile([P, 2 * T], BF16)
            kT2 = attn_res.tile([P, 2 * T], BF16)
            v1_sb = attn_res.tile([P, (T // P) * HPC * VW], BF16)
            nc.vector.memset(v1_sb[:, :], 1.0)
            with tc.tile_pool(name="ph_b_w", bufs=1) as phbw, \
                 tc.tile_pool(name="ph_b", bufs=2) as phb, \
                 tc.tile_pool(name="ph_b_ps", bufs=3, space="PSUM") as phbps:
                wq_all = phbw.tile([P, KB * 3 * DC], BF16, name="wq_all")
                for fc in range(KB):
                    nc.sync.dma_start(
                        out=wq_all[:, fc * 3 * DC:(fc + 1) * 3 * DC],
                        in_=w_qkv[fc * P:(fc + 1) * P, :])
                for t8 in range(NTC):
                    x1c = [phb.tile([P, 8 * TC], BF16, tag=f"x1c{h}",
                                    name=f"x1c{h}") for h in range(2)]
                    for hh in range(2):
                        nc.sync.dma_start(
                            out=x1c[hh][:, :],
                            in_=ag_x1_out[hh][t8 * P:(t8 + 1) * P, :])

                    def xs(fc, lo, sz):
                        hh, fr = fc // 8, fc % 8
                        return x1c[hh][:, fr * TC + lo: fr * TC + lo + sz]

                    for m in range(4):
                        ps = phbps.tile([P, TC], F32, tag="qk", name="ps_qk")
                        for fc in range(KB):
                            nc.tensor.matmul(
                                ps[:, :],
                                wq_all[:, fc * 3 * DC + m * P:
                                       fc * 3 * DC + (m + 1) * P],
                                xs(fc, 0, TC),
                                start=(fc == 0), stop=(fc == KB - 1))
                        dst = qT2 if m < 2 else kT2
                        pair = m % 2
                        off = pair * T + t8 * TC
                        nc.scalar.activation(dst[:, off:off + TC], ps[:, :],
                                             AF.Identity,
                                             bias=bqk_sb[:, m:m + 1])
                    for tt in range(TC // P):
                        psv = phbps.tile([P, DC], F32, tag="v", name="ps_v")
                        for fc in range(KB):
                            nc.tensor.matmul(
                                psv[:, :], xs(fc, tt * P, P),
                                wq_all[:, fc * 3 * DC + 2 * DC:
                                       fc * 3 * DC + 3 * DC],
                                start=(fc == 0), stop=(fc == KB - 1))
                        ttg = t8 * (TC // P) + tt
                        for h in range(HPC):
                            voff = ttg * HPC * VW + h * VW
                            if zero_bv:
                                nc.scalar.activation(
                                    v1_sb[:, voff:voff + HD],
                                    psv[:, h * HD:(h + 1) * HD], AF.Copy)
                            else:
                                nc.vector.tensor_add(
                                    v1_sb[:, voff:voff + HD],
                                    psv[:, h * HD:(h + 1) * HD],
                                    bv_b[:, h * HD:(h + 1) * HD])

            # =========================================================
            # Phase C: attention (local heads, all tokens)
            # den folded into ctx matmul via ones-column of v1
            # =========================================================
            ctx_cm = tc.tile_pool(name="ctx_pool", bufs=1)
            ctx_pool = ctx_cm.__enter__()
            ctxF = [ctx_pool.tile([P, 2 * S], BF16, name=f"ctxF{b}")
                    for b in range(B)]
            with tc.tile_pool(name="ph_c", bufs=1) as phc, \
                 tc.tile_pool(name="ph_c_ps", bufs=1, space="PSUM") as phcps:
                for b in range(B):
                    for qc in range(S // TC):
                        ctx_ps = [phcps.tile([P, TC], F32, tag=f"ctx{h}",
                                             bufs=1, name=f"ctx_ps{h}")
                                  for h in range(HPC)]
                        kts = [kt for kt in range(S // P)
                               if block_status[(kt, qc)] != "skip"]
                        nkt = len(kts)

                        def emit_scores(ki):
                            kt = kts[ki]
                            st = block_status[(kt, qc)]
                            es = []
                            for h in range(HPC):
                                pair, rho = h // 2, h % 2
                                ps_s = phcps.tile([P, TC], F32, tag="s",
                                                  bufs=4, name="ps_s")
                                qoff = pair * T + b * S + qc * TC
                                koff = pair * T + b * S + kt * P
                                nc.tensor.matmul(
                                    ps_s[:, :],
                                    kT2[rho * HD:(rho + 1) * HD,
                                        koff:koff + P],
                                    qT2[rho * HD:(rho + 1) * HD,
                                        qoff:qoff + TC],
                                    start=True, stop=True)
                                e = phc.tile([P, TC], BF16, tag="e", bufs=10,
                                             name="e")
                                nc.scalar.activation(e[:, :], ps_s[:, :],
                                                     AF.Exp,
                                                     scale=1.0 / np.sqrt(HD))
                                if st == "masked":
                                    i = mask_slot[(kt, qc)]
                                    nc.vector.tensor_mul(
                                        e[:, :], e[:, :],
                                        mask_sb[:, i * TC:(i + 1) * TC])
                                es.append(e)
                            return es

                        def emit_ctx(ki, es):
                            kt = kts[ki]
                            ttg = b * (S // P) + kt
                            for h in range(HPC):
                                nc.tensor.matmul(
                                    ctx_ps[h][0:VW, :],
                                    v1_sb[:, ttg * HPC * VW + h * VW:
                                          ttg * HPC * VW + (h + 1) * VW],
                                    es[h][:, :],
                                    start=(ki == 0), stop=(ki == nkt - 1))

                        prev = emit_scores(0)
                        for ki in range(1, nkt):
                            cur = emit_scores(ki)
                            emit_ctx(ki - 1, prev)
                            prev = cur
                        emit_ctx(nkt - 1, prev)

                        for h in range(HPC):
                            pair, rho = h // 2, h % 2
                            den_s = phc.tile([1, TC], F32, tag="dens",
                                             bufs=4, name="den_s")
                            nc.vector.tensor_copy(den_s[:, :],
                                                  ctx_ps[h][HD:HD + 1, :])
                            rec = phc.tile([1, TC], F32, tag="rec", bufs=4,
                                           name="rec")
                            nc.vector.reciprocal(rec[:, :], den_s[:, :])
                            rb = phc.tile([P, TC], F32, tag="rb", bufs=2,
                                          name="rb")
                            nc.gpsimd.partition_broadcast(rb[:, :], rec[:, :])
                            off = qc * 2 * TC + pair * TC
                            hs = slice(rho * HD, (rho + 1) * HD)
                            nc.vector.tensor_mul(ctxF[b][hs, off:off + TC],
                                                 ctx_ps[h][0:HD, :],
                                                 rb[0:HD, :])
                        # ship this (b, qc) chunk to its owner rank slot
                        j = 2 * b + qc
                        for pair in range(2):
                            nc.sync.dma_start(
                                out=a2a_in[j * 2 * P + pair * P:
                                           j * 2 * P + (pair + 1) * P, :],
                                in_=ctxF[b][:, qc * 2 * TC + pair * TC:
                                            qc * 2 * TC + (pair + 1) * TC])
            nc.gpsimd.collective_compute(
                "AllToAll", ALU.bypass, replica_groups=RG,
                ins=[a2a_in[:, :].opt()], outs=[a2a_out[:, :].opt()])
            ctx_cm.__exit__(None, None, None)
            attn_res_cm.__exit__(None, None, None)

            # =========================================================
            # Phase D: dense + LN3 + residual + LN2 (all local tokens)
            # =========================================================
            def ln_rows(pool, ps_s, ps_q, name):
                mu = pool.tile([1, TC], F32, tag="lr_mu", name=f"{name}_mu")
                m2 = pool.tile([1, TC], F32, tag="lr_m2", name=f"{name}_m2")
                var = pool.tile([1, TC], F32, tag="lr_var", name=f"{name}_v")
                sd = pool.tile([1, TC], F32, tag="lr_sd", name=f"{name}_sd")
                a_row = pool.tile([1, TC], F32, tag="lr_a", name=f"{name}_a")
                b2_row = pool.tile([1, TC], F32, tag="lr_b2",
                                   name=f"{name}_b2")
                nc.vector.tensor_scalar_mul(mu[:, :], ps_s[:, :], 1.0 / H)
                nc.vector.tensor_scalar_mul(m2[:, :], ps_q[:, :], 1.0 / H)
                nc.vector.tensor_mul(var[:, :], mu[:, :], mu[:, :])
                nc.vector.tensor_sub(var[:, :], m2[:, :], var[:, :])
                nc.scalar.activation(sd[:, :], var[:, :], AF.Sqrt, bias=EPS)
                nc.vector.reciprocal(a_row[:, :], sd[:, :])
                nc.vector.tensor_mul(b2_row[:, :], mu[:, :], a_row[:, :])
                nc.vector.tensor_scalar_mul(b2_row[:, :], b2_row[:, :], -1.0)
                a_b = pool.tile([P, TC], F32, tag="lr_ab", name=f"{name}_ab")
                b2_b = pool.tile([P, TC], F32, tag="lr_bb", name=f"{name}_bb")
                nc.gpsimd.partition_broadcast(a_b[:, :], a_row[:, :])
                nc.gpsimd.partition_broadcast(b2_b[:, :], b2_row[:, :])
                return a_b, b2_b

            with tc.tile_pool(name="ph_d", bufs=1) as phd, \
                 tc.tile_pool(name="ph_d_ps", bufs=1, space="PSUM") as phdps:
                attn_sb = phd.tile([P, KB * TC], BF16, name="attn_sb")
                ctx_my = phd.tile([P, KB * TC], BF16, name="ctx_my")
                for kb in range(KB):
                    nc.sync.dma_start(out=ctx_my[:, kb * TC:(kb + 1) * TC],
                                      in_=a2a_out[kb * P:(kb + 1) * P, :])
                ps_s3 = phdps.tile([1, TC], F32, tag="st_s", bufs=2,
                                   name="ps_s3")
                ps_q3 = phdps.tile([1, TC], F32, tag="st_q", bufs=2,
                                   name="ps_q3")
                for pss in range(4):
                    dps = [phdps.tile([P, TC], F32, tag=f"d{i}", bufs=1,
                                      name=f"dps{i}") for i in range(4)]
                    for kb in range(KB):
                        wd_c = phd.tile([P, 4 * P], BF16, tag="wd", bufs=4,
                                        name="wd_c")
                        nc.sync.dma_start(
                            out=wd_c[:, :],
                            in_=w_dense[kb * P:(kb + 1) * P,
                                        pss * 4 * P:(pss + 1) * 4 * P])
                        for i in range(4):
                            nc.tensor.matmul(
                                dps[i][:, :], wd_c[:, i * P:(i + 1) * P],
                                ctx_my[:, kb * TC:(kb + 1) * TC],
                                start=(kb == 0), stop=(kb == KB - 1))
                    for i in range(4):
                        mb = pss * 4 + i
                        sl = slice(mb * TC, (mb + 1) * TC)
                        nc.scalar.activation(attn_sb[:, sl], dps[i][:, :],
                                             AF.Identity,
                                             bias=bdense_sb[:, mb:mb + 1])
                        nc.tensor.matmul(ps_s3[:, :], ones_bf[:, 0:1],
                                         attn_sb[:, sl],
                                         start=(mb == 0), stop=(mb == KB - 1))
                        sq = phd.tile([P, TC], BF16, tag="sq", bufs=3,
                                      name="sq")
                        nc.vector.tensor_mul(sq[:, :], attn_sb[:, sl],
                                             attn_sb[:, sl])
                        nc.tensor.matmul(ps_q3[:, :], ones_bf[:, 0:1],
                                         sq[:, :],
                                         start=(mb == 0), stop=(mb == KB - 1))
                a3_b, b23_b = ln_rows(phd, ps_s3, ps_q3, "ln3")
                ps_s2 = phdps.tile([1, TC], F32, tag="st_s", bufs=2,
                                   name="ps_s2")
                ps_q2 = phdps.tile([1, TC], F32, tag="st_q", bufs=2,
                                   name="ps_q2")
                for mb in range(KB):
                    sl = slice(mb * TC, (mb + 1) * TC)
                    t1 = phd.tile([P, TC], F32, tag="t1", bufs=2, name="t1")
                    nc.vector.tensor_mul(t1[:, :], attn_sb[:, sl], a3_b[:, :])
                    nc.vector.tensor_add(t1[:, :], t1[:, :], b23_b[:, :])
                    if not ln_triv:
                        nc.vector.tensor_scalar(t1[:, :], t1[:, :],
                                                ln3w_sb[:, mb:mb + 1],
                                                ln3b_sb[:, mb:mb + 1],
                                                ALU.mult, ALU.add)
                    nc.vector.tensor_add(ln_in[:, sl], t1[:, :], h_bf[:, sl])
                    nc.tensor.matmul(ps_s2[:, :], ones_bf[:, 0:1],
                                     ln_in[:, sl],
                                     start=(mb == 0), stop=(mb == KB - 1))
                    sq = phd.tile([P, TC], BF16, tag="sq", bufs=3, name="sq")
                    nc.vector.tensor_mul(sq[:, :], ln_in[:, sl],
                                         ln_in[:, sl])
                    nc.tensor.matmul(ps_q2[:, :], ones_bf[:, 0:1], sq[:, :],
                                     start=(mb == 0), stop=(mb == KB - 1))
                a2_b, b22_b = ln_rows(phd, ps_s2, ps_q2, "ln2")
                for mb in range(KB):
                    sl = slice(mb * TC, (mb + 1) * TC)
                    t1 = phd.tile([P, TC], F32, tag="t1", bufs=2, name="t1")
                    nc.vector.tensor_mul(t1[:, :], ln_in[:, sl], a2_b[:, :])
                    if ln_triv:
                        nc.vector.tensor_add(x2[:, sl], t1[:, :],
                                             b22_b[:, :])
                    else:
                        nc.vector.tensor_add(t1[:, :], t1[:, :], b22_b[:, :])
                        nc.vector.tensor_scalar(x2[:, sl], t1[:, :],
                                                ln2w_sb[:, mb:mb + 1],
                                                ln2b_sb[:, mb:mb + 1],
                                                ALU.mult, ALU.add)

            # =========================================================
            # Phase E: h4h + gelu (local tokens, streamed weights)
            # =========================================================
            inter_cm = tc.tile_pool(name="inter_pool", bufs=1)
            interp = inter_cm.__enter__()
            inter = interp.tile([P, KB4 * TC], BF16, name="inter")
            with tc.tile_pool(name="ph_e", bufs=1) as phe, \
                 tc.tile_pool(name="ph_e_ps", bufs=1, space="PSUM") as pheps:
                for pss in range(16):
                    hps = [pheps.tile([P, TC], F32, tag=f"h{i}", bufs=2,
                                      name=f"hps{i}") for i in range(4)]
                    for kb in range(KB):
                        wh_c = phe.tile([P, 4 * P], BF16, tag="wh", bufs=4,
                                        name="wh_c")
                        nc.sync.dma_start(
                            out=wh_c[:, :],
                            in_=w_h4h[kb * P:(kb + 1) * P,
                                      pss * 4 * P:(pss + 1) * 4 * P])
                        for i in range(4):
                            nc.tensor.matmul(
                                hps[i][:, :], wh_c[:, i * P:(i + 1) * P],
                                x2[:, kb * TC:(kb + 1) * TC],
                                start=(kb == 0), stop=(kb == KB - 1))
                    for i in range(4):
                        fb = pss * 4 + i
                        nc.scalar.activation(
                            inter[:, fb * TC:(fb + 1) * TC], hps[i][:, :],
                            AF.Gelu_apprx_tanh, bias=bh4h_sb[:, fb:fb + 1])

            # =========================================================
            # Phase F: 4hh + LN4 + residual -> out (local tokens)
            # =========================================================
            with tc.tile_pool(name="ph_f", bufs=1) as phf, \
                 tc.tile_pool(name="ph_f_ps", bufs=1, space="PSUM") as phfps:
                mlp_sb = phf.tile([P, KB * TC], BF16, name="mlp_sb")
                ps_s4 = phfps.tile([1, TC], F32, tag="st_s", bufs=1,
                                   name="ps_s4")
                ps_q4 = phfps.tile([1, TC], F32, tag="st_q", bufs=1,
                                   name="ps_q4")
                for pss in range(4):
                    fps = [phfps.tile([P, TC], F32, tag=f"f{i}", bufs=1,
                                      name=f"fps{i}") for i in range(4)]
                    for kb in range(KB4):
                        w4_c = phf.tile([P, 4 * P], BF16, tag="w4", bufs=4,
                                        name="w4_c")
                        nc.sync.dma_start(
                            out=w4_c[:, :],
                            in_=w_4hh[kb * P:(kb + 1) * P,
                                      pss * 4 * P:(pss + 1) * 4 * P])
                        for i in range(4):
                            nc.tensor.matmul(
                                fps[i][:, :], w4_c[:, i * P:(i + 1) * P],
                                inter[:, kb * TC:(kb + 1) * TC],
                                start=(kb == 0), stop=(kb == KB4 - 1))
                    for i in range(4):
                        mb = pss * 4 + i
                        sl = slice(mb * TC, (mb + 1) * TC)
                        nc.scalar.activation(mlp_sb[:, sl], fps[i][:, :],
                                             AF.Identity,
                                             bias=b4hh_sb[:, mb:mb + 1])
                        nc.tensor.matmul(ps_s4[:, :], ones_bf[:, 0:1],
                                         mlp_sb[:, sl],
                                         start=(mb == 0), stop=(mb == KB - 1))
                        sq = phf.tile([P, TC], BF16, tag="sq", bufs=3,
                                      name="sq")
                        nc.vector.tensor_mul(sq[:, :], mlp_sb[:, sl],
                                             mlp_sb[:, sl])
                        nc.tensor.matmul(ps_q4[:, :], ones_bf[:, 0:1],
                                         sq[:, :],
                                         start=(mb == 0), stop=(mb == KB - 1))
                a4_b, b24_b = ln_rows(phf, ps_s4, ps_q4, "ln4")
                for mb in range(KB):
                    sl = slice(mb * TC, (mb + 1) * TC)
                    t1 = phf.tile([P, TC], F32, tag="t1", bufs=2, name="t1")
                    nc.vector.tensor_mul(t1[:, :], mlp_sb[:, sl], a4_b[:, :])
                    nc.vector.tensor_add(t1[:, :], t1[:, :], b24_b[:, :])
                    if not ln_triv:
                        nc.vector.tensor_scalar(t1[:, :], t1[:, :],
                                                ln4w_sb[:, mb:mb + 1],
                                                ln4b_sb[:, mb:mb + 1],
                                                ALU.mult, ALU.add)
                    ot = phf.tile([P, TC], F32, tag="ot", bufs=2, name="ot")
                    nc.vector.tensor_add(ot[:, :], t1[:, :], ln_in[:, sl])
                    nc.sync.dma_start(
                        out=out_ext[mb * P:(mb + 1) * P, :], in_=ot[:, :])
            inter_cm.__exit__(None, None, None)

    nc.compile()
    return nc


# ----------------------------------------------------------------------
_cache = {}


def _get_program(mask_np, zero_bv, ln_triv):
    key = (mask_np.tobytes(), zero_bv, ln_triv)
    kh = hash(key)
    if kh not in _cache:
        _cache[kh] = build_program(_causal_block_status(mask_np), zero_bv,
                                   ln_triv)
    return _cache[kh]


def kernel(hidden_states, mask, ln1_w, ln1_b, w_qkv, b_qkv, w_dense, b_dense,
           ln3_w, ln3_b, ln2_w, ln2_b, w_h4h, b_h4h, w_4hh, b_4hh,
           ln4_w, ln4_b):
    hidden_states = np.asarray(hidden_states, np.float32)
    mask2d = np.asarray(mask, np.float32).reshape(S, S)
    w_qkv = np.asarray(w_qkv, np.float32)
    b_qkv = np.asarray(b_qkv, np.float32)

    zero_bv = bool(np.all(b_qkv[2 * H:] == 0.0))
    ln_triv = all(
        bool(np.all(np.asarray(w) == 1.0)) and
        bool(np.all(np.asarray(b) == 0.0))
        for w, b in [(ln1_w, ln1_b), (ln2_w, ln2_b), (ln3_w, ln3_b),
                     (ln4_w, ln4_b)])
    prog = _get_program(mask2d, zero_bv, ln_triv)

    hT = np.ascontiguousarray(hidden_states.reshape(T, H).T)
    maskT_bf = np.ascontiguousarray(mask2d.T).astype(bf16)
    wd_bf = np.ascontiguousarray(np.asarray(w_dense, np.float32)).astype(bf16)
    wh_bf = np.ascontiguousarray(np.asarray(w_h4h, np.float32)).astype(bf16)
    w4_bf = np.ascontiguousarray(np.asarray(w_4hh, np.float32)).astype(bf16)
    ln1w = np.asarray(ln1_w, np.float32).reshape(H, 1)
    ln1b = np.asarray(ln1_b, np.float32).reshape(H, 1)
    ln2w = np.asarray(ln2_w, np.float32).reshape(H, 1)
    ln2b = np.asarray(ln2_b, np.float32).reshape(H, 1)
    ln3w = np.asarray(ln3_w, np.float32).reshape(H, 1)
    ln3b = np.asarray(ln3_b, np.float32).reshape(H, 1)
    ln4w = np.asarray(ln4_w, np.float32).reshape(H, 1)
    ln4b = np.asarray(ln4_b, np.float32).reshape(H, 1)
    bd = np.asarray(b_dense, np.float32).reshape(H, 1)
    bh = np.asarray(b_h4h, np.float32).reshape(F4, 1)
    b4 = np.asarray(b_4hh, np.float32).reshape(H, 1)

    in_maps = []
    for c in range(NC):
        wq_c = np.concatenate([w_qkv[:, c * DC:(c + 1) * DC],
                               w_qkv[:, H + c * DC:H + (c + 1) * DC],
                               w_qkv[:, 2 * H + c * DC:2 * H + (c + 1) * DC]],
                              axis=1)
        b_qk_c = np.concatenate([b_qkv[c * DC:(c + 1) * DC],
                                 b_qkv[H + c * DC:H + (c + 1) * DC]])
        b_v_c = b_qkv[2 * H + c * DC:2 * H + (c + 1) * DC]
        im = {
            "h_ln1": np.ascontiguousarray(hT[:, c * TC:(c + 1) * TC]),
            "ln1_w": ln1w, "ln1_b": ln1b,
            "ln2_w": ln2w, "ln2_b": ln2b,
            "ln3_w": ln3w, "ln3_b": ln3b,
            "ln4_w": ln4w, "ln4_b": ln4b,
            "w_qkv": np.ascontiguousarray(wq_c).astype(bf16),
            "b_qk": np.ascontiguousarray(b_qk_c).reshape(2 * DC, 1),
            "b_v": np.ascontiguousarray(b_v_c).reshape(1, DC),
            "w_dense": wd_bf, "b_dense": bd,
            "w_h4h": wh_bf, "b_h4h": bh,
            "w_4hh": w4_bf, "b_4hh": b4,
            "maskT": maskT_bf,
        }
        in_maps.append(im)

    res = run_bass_kernel_spmd(prog, in_maps, core_ids=list(range(NC)))
    outT = np.concatenate([res.results[c]["out"] for c in range(NC)], axis=1)
    return np.ascontiguousarray(outT.T).reshape(B, S, H).astype(np.float32)
